# revision 13
# baseline (speedup 1.0000x reference)
"""MoE ConditionalFeedForward (SwiGLU, T=2048 D=1024 I=4096 E=8 K=2) on 8 TRN2 cores.

Strategy: expert-parallel, one expert per NeuronCore. Routing/gather happens on
host (numpy): for each expert e, collect the unique tokens routed to it, merge
the two top-k gate weights, and ship the gathered tokens transposed plus that
expert's three weight matrices, pre-packed bf16 so every device DMA is a
contiguous HBM read. The per-token gate is folded into the w3 branch ON CHIP:
a K=1 outer-product matmul broadcasts the gate row across partitions and DVE
multiplies produce x3 = g*x, so no second x tensor rides the startup DMAs.
Each core computes
  yT_e = w2e_T-chunks @ (silu(x @ w1e^T) * ((g*x) @ w3e^T))^T
for its <=N_TOK tokens; the host scatter-adds the 8 transposed partials.

Device kernel (per core), all matmul operands bf16 (accuracy ~5e-3 absmax-rel
vs the 2e-2 gate; halves HBM traffic vs fp32r and enables the PE's
fast-weight-load path, which removes the ~55ns/matmul LDWEIGHTS tax):
  warmup:  dummy N=128 matmuls on a memset tile while the startup DMAs land,
           so the HAM clock-gate un-throttles (1.2->2.4 GHz) before the real
           stream begins.
  layer 1: per i-tile, accumulate 8 K=128 steps into two PSUM banks (h1, h3),
           then ACT silu + DVE multiply into an SBUF hT tile laid out [i, t]
           so it feeds layer 2 as the moving operand directly. The first
           matmul gates on a 32 KB duplicated first-chunk tensor; every DMA
           is a contiguous HBM range (big linear transfers are auto-sprayed
           across all 16 DMA rings). No GPSIMD (slow boot + slow SWDGE
           descriptor issue). w2 streams into a resident SBUF tile during
           layer 1, delayed so it never competes with ramp-critical w13
           loads. Matmul free dim is N_TOK=504 (>= max tokens routed to one
           expert at this problem size), not the padded 512.
  layer 2: d-tile-major, transposed: yT[d, t] for one 128-row d-tile
           accumulates over all 32 i-tiles into 1 PSUM bank (lhsT = resident
           w2 chunk, moving = hT), then one ACT/DVE copy drains it and a
           single linear 128 KB DMA ships it while the next d-tile computes;
           only the last tile's drain is exposed.
"""

import math
import os
import sys
import time
import types

for _p in ("/opt/trn_rl_repo", "/opt/pypackages"):
    if _p not in sys.path:
        sys.path.append(_p)

import numpy as np

# antenv.axon_hooks is absent from this image; run_bass_kernel_spmd imports it
# unconditionally when tracing is requested (BASS_TRACE=1). Provide the
# documented shim so profiling works when asked for and degrades to a no-op
# otherwise. No-op if a real antenv.axon_hooks exists.
def _ensure_ntff_hook():
    try:
        import antenv
    except ImportError:
        return
    try:
        import antenv.axon_hooks  # noqa: F401
        return
    except ImportError:
        pass
    mod = types.ModuleType("antenv.axon_hooks")
    mod._hook = None

    def set_axon_ntff_profile_hook(h):
        mod._hook = h

    def get_axon_ntff_profile_hook():
        if mod._hook is None:
            try:
                from trn_agent_boot.trn_boot import _ntff_profile_via_ctypes

                mod._hook = _ntff_profile_via_ctypes("/opt/axon/libaxon_pjrt.so")
            except Exception:
                mod._hook = None
        return mod._hook

    mod.set_axon_ntff_profile_hook = set_axon_ntff_profile_hook
    mod.get_axon_ntff_profile_hook = get_axon_ntff_profile_hook
    sys.modules["antenv.axon_hooks"] = mod
    antenv.axon_hooks = mod


_ensure_ntff_hook()

import concourse.bacc as bacc
import concourse.tile as tile
from concourse import mybir
from concourse.bass_utils import run_bass_kernel_spmd

T, D, I, E, TOPK = 2048, 1024, 4096, 8, 2
N_CORES = 8
CAP = 512            # token tile capacity (multiple of 128, <=512)
N_TOK = 504          # matmul free dim: tokens actually computed per pass
DT = D // 128        # 8 contraction steps for layer 1
ND = D // 128        # 8 output d-tiles for layer 2
NI = I // 128        # 32 intermediate tiles
N_WARM = 28          # dummy matmuls to lift the HAM clock gate at startup
F32 = mybir.dt.float32
BF16 = mybir.dt.bfloat16
NP_BF16 = mybir.dt.np(BF16)

_NC = None           # compiled Bass module, built once per process
_WCACHE = {}         # packed per-expert weights, keyed on input identity
LAST_RESULTS = None  # BassKernelResults of the most recent SPMD run


def _build_nc(sim_act=False):
    # sim_act: CoreSim lacks Silu; emit sigmoid + extra multiply instead
    # (same math) so the program can be validated in simulation.
    nc = bacc.Bacc(
        "TRN2", target_bir_lowering=False, debug=False, num_devices=N_CORES
    )
    # Packed layouts (see _pack_weights): every DMA below is a contiguous
    # HBM range.
    xt1_d = nc.dram_tensor("xt1", [DT, 128, CAP], BF16, kind="ExternalInput").ap()
    g_d = nc.dram_tensor("g", [1, CAP], BF16, kind="ExternalInput").ap()
    w13f_d = nc.dram_tensor(
        "w13f", [2, DT, 128, 128], BF16, kind="ExternalInput"
    ).ap()
    w13p_d = nc.dram_tensor(
        "w13p", [NI, 2, 128, DT, 128], BF16, kind="ExternalInput"
    ).ap()
    w2t_d = nc.dram_tensor("w2t", [I, D], BF16, kind="ExternalInput").ap()
    y_d = nc.dram_tensor("y", [D, CAP], BF16, kind="ExternalOutput").ap()

    with tile.TileContext(nc) as tc:
        with (
            tc.tile_pool(name="consts", bufs=1) as const_pool,
            tc.tile_pool(name="w13", bufs=10) as w13_pool,
            tc.tile_pool(name="h", bufs=1) as h_pool,
            tc.tile_pool(name="tmp", bufs=2) as tmp_pool,
            tc.tile_pool(name="yout", bufs=4) as out_pool,
        ):
            # HAM warmup + gate-broadcast seeds (no DMA deps except tiny g).
            warm_sb = const_pool.tile([128, 128], BF16)
            nc.vector.memset(warm_sb[:], 0.0)
            ones_sb = const_pool.tile([1, 128], BF16)
            nc.vector.memset(ones_sb[:], 1.0)
            g_row = const_pool.tile([1, CAP], BF16)

            # Startup-critical loads, issued in PE consumption order so each
            # piece lands just in time: i-tile 0's w1/w3 come from a
            # duplicated per-dt-chunk tensor (every chunk a 32 KB linear
            # read), interleaved with the xt1 d-tiles (needed by both the h1
            # matmuls and the on-chip x3 build).
            xt1_sb = const_pool.tile([128, DT, CAP], BF16)
            xt3_sb = const_pool.tile([128, DT, CAP], BF16)
            w13_t0 = w13_pool.tile([128, 2, DT, 128], BF16, tag="w13")
            nc.sync.dma_start(g_row[:], g_d)
            startup = []
            for dt_i in range(DT):
                startup.append(("m0", dt_i))
                startup.append(("xt", dt_i))
                if dt_i >= 2:
                    startup.append(("m1", dt_i - 2))
            startup += [("m1", DT - 2), ("m1", DT - 1)]
            for kind, dt_i in startup:
                if kind == "xt":
                    nc.sync.dma_start(xt1_sb[:, dt_i, :], xt1_d[dt_i])
                else:
                    m = 0 if kind == "m0" else 1
                    nc.sync.dma_start(
                        w13_t0[:, m, dt_i, :], w13f_d[m, dt_i]
                    )

            # hT[i, t] — layer-1 output (gate folded in via the x3 branch),
            # transposed so it is the layer-2 moving operand directly.
            hT = h_pool.tile([128, NI, CAP], BF16)

            # w2 lives fully in SBUF (64 KB/partition bf16); its i-tiles are
            # streamed in during layer 1 so layer 2 never waits on HBM.
            w2_sb = const_pool.tile([128, NI, D], BF16)
            w2t_r = w2t_d.rearrange("(a p) d -> p a d", p=128)

            # Warmup matmuls, then the gate outer product: g_bc[p, t] =
            # ones[p] * g[t] via a K=1 matmul, copied to SBUF by DVE (ACT is
            # busy with its activation-table loads at startup).
            warm_pool = tc.alloc_tile_pool(name="warm", bufs=1, space="PSUM")
            warm_ps = warm_pool.tile([128, 128], F32)
            for _ in range(N_WARM):
                nc.tensor.matmul(
                    warm_ps[:], warm_sb[:], warm_sb[:], start=True, stop=True
                )
            gps_pool = tc.alloc_tile_pool(name="gps", bufs=1, space="PSUM")
            g_ps = gps_pool.tile([128, CAP], F32)
            nc.tensor.matmul(g_ps[:], ones_sb[:], g_row[:], start=True, stop=True)
            g_bc = const_pool.tile([128, CAP], BF16)
            nc.vector.tensor_copy(g_bc[:], g_ps[:])
            gps_pool.release()
            warm_pool.release()
            # x3 = g * x, built per d-tile as its xt1 slice lands.
            for dt_i in range(DT):
                nc.vector.tensor_mul(
                    xt3_sb[:, dt_i, :N_TOK],
                    xt1_sb[:, dt_i, :N_TOK],
                    g_bc[:, :N_TOK],
                )

            ps1_pool = tc.alloc_tile_pool(name="ps1", bufs=2, space="PSUM")
            ps2_pool = tc.alloc_tile_pool(name="ps2", bufs=2, space="PSUM")
            for it in range(NI):
                if it == 0:
                    w13_t = w13_t0
                else:
                    w13_t = w13_pool.tile([128, 2, DT, 128], BF16, tag="w13")
                    for m in range(2):
                        nc.sync.dma_start(w13_t[:, m], w13p_d[it, m])
                # Stream one w2 i-tile per iteration into the resident tile,
                # delayed so it never queues ahead of ramp-critical w13.
                if it >= 6:
                    nc.sync.dma_start(w2_sb[:, it - 6, :], w2t_r[:, it - 6, :])
                w1_t = w13_t[:, 0]
                w3_t = w13_t[:, 1]

                h1_ps = ps1_pool.tile([128, CAP], F32, tag="h1")
                h3_ps = ps1_pool.tile([128, CAP], F32, tag="h3")
                for dt_i in range(DT):
                    nc.tensor.matmul(
                        h1_ps[:, :N_TOK],
                        w1_t[:, dt_i, :],
                        xt1_sb[:, dt_i, :N_TOK],
                        start=(dt_i == 0),
                        stop=(dt_i == DT - 1),
                    )
                for dt_i in range(DT):
                    nc.tensor.matmul(
                        h3_ps[:, :N_TOK],
                        w3_t[:, dt_i, :],
                        xt3_sb[:, dt_i, :N_TOK],
                        start=(dt_i == 0),
                        stop=(dt_i == DT - 1),
                    )
                s_sb = tmp_pool.tile([128, N_TOK], F32)
                if sim_act:
                    nc.scalar.activation(
                        s_sb[:], h1_ps[:, :N_TOK],
                        mybir.ActivationFunctionType.Sigmoid,
                    )
                    nc.vector.tensor_mul(s_sb[:], s_sb[:], h1_ps[:, :N_TOK])
                else:
                    nc.scalar.activation(
                        s_sb[:], h1_ps[:, :N_TOK],
                        mybir.ActivationFunctionType.Silu,
                    )
                nc.vector.tensor_mul(
                    hT[:, it, :N_TOK], s_sb[:], h3_ps[:, :N_TOK]
                )
            # Remaining w2 i-tiles (land well before layer 2 needs them).
            for it in range(NI - 6, NI):
                nc.sync.dma_start(w2_sb[:, it, :], w2t_r[:, it, :])

            # Layer 2, d-tile-major, transposed: yT[d, t] accumulates over
            # all 32 i-tiles into 1 PSUM bank per d-tile; drain is a single
            # ACT/DVE copy + one linear 128 KB DMA, pipelined 2 deep.
            for dd in range(ND):
                yT_ps = ps2_pool.tile([128, 512], F32, tag="yt")
                for it in range(NI):
                    nc.tensor.matmul(
                        yT_ps[:, :N_TOK],
                        w2_sb[:, it, dd * 128:(dd + 1) * 128],
                        hT[:, it, :N_TOK],
                        start=(it == 0),
                        stop=(it == NI - 1),
                    )
                yT_sb = out_pool.tile([128, CAP], BF16)
                if dd == ND - 1:
                    # Exposed tail: split the drain across both engines by
                    # column range, each half's DMA starting as soon as its
                    # own copy lands.
                    half = N_TOK // 2
                    nc.scalar.activation(
                        yT_sb[:, :half], yT_ps[:, :half],
                        mybir.ActivationFunctionType.Copy,
                    )
                    nc.sync.dma_start(
                        y_d[dd * 128:(dd + 1) * 128, :half], yT_sb[:, :half]
                    )
                    nc.vector.tensor_copy(
                        yT_sb[:, half:N_TOK], yT_ps[:, half:N_TOK]
                    )
                    nc.sync.dma_start(
                        y_d[dd * 128:(dd + 1) * 128, half:N_TOK],
                        yT_sb[:, half:N_TOK],
                    )
                else:
                    if dd % 2 == 0:
                        nc.scalar.activation(
                            yT_sb[:, :N_TOK], yT_ps[:, :N_TOK],
                            mybir.ActivationFunctionType.Copy,
                        )
                    else:
                        nc.vector.tensor_copy(yT_sb[:, :N_TOK], yT_ps[:, :N_TOK])
                    nc.sync.dma_start(
                        y_d[dd * 128:(dd + 1) * 128, :], yT_sb[:]
                    )
            ps2_pool.release()
            ps1_pool.release()

    nc.compile()
    return nc


def _pack_weights(w1, w2, w3):
    """Per-expert device layouts (bf16), all contiguous HBM reads:
    w13p[it, m, p, dt, c] = w_m[it*128+c, dt*128+p]  (w.T tiled for lhsT;
    each (it, m) plane is one contiguous 256 KB read, 2 KB per partition)
    w13f[m, dt, p, c] = w13p[0, m, p, dt, c]  (i-tile 0 duplicated as per-dt
    32 KB linear chunks for the startup ramp)
    w2t = w2.T ([I, D], i rows on partitions)."""
    key = tuple((a.ctypes.data, a.shape) for a in (w1, w2, w3))
    if _WCACHE.get("key") == key:
        return _WCACHE["maps"]
    maps = []
    for e in range(E):
        w13p = np.empty((NI, 2, 128, DT, 128), dtype=NP_BF16)
        # w[e] is [I, D]: reshape [it, c, dt, p] -> transpose to [it, p, dt, c]
        w13p[:, 0] = w1[e].reshape(NI, 128, DT, 128).transpose(0, 3, 2, 1).astype(NP_BF16)
        w13p[:, 1] = w3[e].reshape(NI, 128, DT, 128).transpose(0, 3, 2, 1).astype(NP_BF16)
        w13f = np.ascontiguousarray(w13p[0].transpose(0, 2, 1, 3))
        w2t = np.ascontiguousarray(w2[e].T).astype(NP_BF16)
        maps.append({"w13p": w13p, "w13f": w13f, "w2t": w2t})
    _WCACHE["key"] = key
    _WCACHE["maps"] = maps
    return maps


def kernel(x, expert_indices, expert_weights, w1, w2, w3):
    global _NC, LAST_RESULTS
    x = np.ascontiguousarray(np.asarray(x, dtype=np.float32))
    idx = np.asarray(expert_indices)
    ew = np.asarray(expert_weights, dtype=np.float32)
    w1 = np.ascontiguousarray(np.asarray(w1, dtype=np.float32))
    w2 = np.ascontiguousarray(np.asarray(w2, dtype=np.float32))
    w3 = np.ascontiguousarray(np.asarray(w3, dtype=np.float32))

    if _NC is None:
        _NC = _build_nc()

    # Host routing: unique tokens per expert, with both top-k gate weights of a
    # token merged (a token picking the same expert twice gets the summed gate).
    tok_lists, gate_lists = [], []
    for e in range(E):
        m = idx == e
        sel = np.nonzero(m.any(axis=1))[0]
        tok_lists.append(sel)
        gate_lists.append((ew * m).sum(axis=1)[sel].astype(np.float32))

    weight_maps = _pack_weights(w1, w2, w3)

    n_pass = max(1, math.ceil(max(len(s) for s in tok_lists) / N_TOK))
    out = np.zeros((T, D), dtype=np.float32)
    trace = bool(os.environ.get("BASS_TRACE"))
    for p in range(n_pass):
        in_maps = []
        chunks = []
        for e in range(E):
            sel = tok_lists[e][p * N_TOK:(p + 1) * N_TOK]
            g = gate_lists[e][p * N_TOK:(p + 1) * N_TOK]
            chunks.append(sel)
            xt1 = np.zeros((DT, 128, CAP), dtype=NP_BF16)
            g_pad = np.zeros((1, CAP), dtype=NP_BF16)
            if len(sel):
                xt1.reshape(D, CAP)[:, :len(sel)] = x[sel].T.astype(NP_BF16)
                g_pad[0, :len(sel)] = g.astype(NP_BF16)
            in_maps.append({"xt1": xt1, "g": g_pad, **weight_maps[e]})
        # Rare transient NRT_EXEC_UNIT_UNRECOVERABLE errors have been observed
        # on the first execution of a fresh NEFF; a straight retry recovers.
        last_exc = None
        for attempt in range(3):
            try:
                LAST_RESULTS = run_bass_kernel_spmd(
                    _NC, in_maps, core_ids=list(range(N_CORES)),
                    trace=trace and attempt == 0,
                )
                break
            except Exception as exc:  # noqa: BLE001
                last_exc = exc
                time.sleep(3)
        else:
            raise last_exc
        for e in range(E):
            sel = chunks[e]
            if len(sel):
                out[sel] += (
                    LAST_RESULTS.results[e]["y"][:, :len(sel)]
                    .T.astype(np.float32)
                )
    return out


# revision 14
# speedup vs baseline: 1.2429x; 1.2429x over previous
"""MoE ConditionalFeedForward (SwiGLU, T=2048 D=1024 I=4096 E=8 K=2) on 8 TRN2 cores.

Strategy: expert-parallel, one expert per NeuronCore. Routing/gather happens on
host (numpy): for each expert e, collect the unique tokens routed to it, merge
the two top-k gate weights, and ship the gathered tokens transposed plus that
expert's three weight matrices, pre-packed bf16 so every device DMA is a
contiguous HBM read. The per-token gate is folded into the w3 branch ON CHIP:
a K=1 outer-product matmul broadcasts the gate row across partitions and DVE
multiplies produce x3 = g*x, so no second x tensor rides the startup DMAs.
Each core computes
  yT_e = w2e_T-chunks @ (silu(x @ w1e^T) * ((g*x) @ w3e^T))^T
for its <=N_TOK tokens; the host scatter-adds the 8 transposed partials.

Device kernel (per core), all matmul operands bf16 (accuracy ~5e-3 absmax-rel
vs the 2e-2 gate; halves HBM traffic vs fp32r and enables the PE's
fast-weight-load path, which removes the ~55ns/matmul LDWEIGHTS tax):
  warmup:  dummy N=128 matmuls on a memset tile while the startup DMAs land,
           so the HAM clock-gate un-throttles (1.2->2.4 GHz) before the real
           stream begins.
  layer 1: per i-tile, accumulate 8 K=128 steps into two PSUM banks (h1, h3),
           then ACT silu + DVE multiply into an SBUF hT tile laid out [i, t]
           so it feeds layer 2 as the moving operand directly. The first
           matmul gates on a 32 KB duplicated first-chunk tensor; every DMA
           is a contiguous HBM range (big linear transfers are auto-sprayed
           across all 16 DMA rings). No GPSIMD (slow boot + slow SWDGE
           descriptor issue). w2 streams into a resident SBUF tile during
           layer 1, delayed so it never competes with ramp-critical w13
           loads. Matmul free dim is N_TOK=504 (>= max tokens routed to one
           expert at this problem size), not the padded 512.
  layer 2: d-tile-major, transposed: yT[d, t] for one 128-row d-tile
           accumulates over all 32 i-tiles into 1 PSUM bank (lhsT = resident
           w2 chunk, moving = hT), then one ACT/DVE copy drains it and a
           single linear 128 KB DMA ships it while the next d-tile computes;
           only the last tile's drain is exposed.
"""

import math
import os
import sys
import time
import types

for _p in ("/opt/trn_rl_repo", "/opt/pypackages"):
    if _p not in sys.path:
        sys.path.append(_p)

import numpy as np

# antenv.axon_hooks is absent from this image; run_bass_kernel_spmd imports it
# unconditionally when tracing is requested (BASS_TRACE=1). Provide the
# documented shim so profiling works when asked for and degrades to a no-op
# otherwise. No-op if a real antenv.axon_hooks exists.
def _ensure_ntff_hook():
    try:
        import antenv
    except ImportError:
        return
    try:
        import antenv.axon_hooks  # noqa: F401
        return
    except ImportError:
        pass
    mod = types.ModuleType("antenv.axon_hooks")
    mod._hook = None

    def set_axon_ntff_profile_hook(h):
        mod._hook = h

    def get_axon_ntff_profile_hook():
        if mod._hook is None:
            try:
                from trn_agent_boot.trn_boot import _ntff_profile_via_ctypes

                mod._hook = _ntff_profile_via_ctypes("/opt/axon/libaxon_pjrt.so")
            except Exception:
                mod._hook = None
        return mod._hook

    mod.set_axon_ntff_profile_hook = set_axon_ntff_profile_hook
    mod.get_axon_ntff_profile_hook = get_axon_ntff_profile_hook
    sys.modules["antenv.axon_hooks"] = mod
    antenv.axon_hooks = mod


_ensure_ntff_hook()

import concourse.bacc as bacc
import concourse.tile as tile
from concourse import mybir
from concourse.bass_utils import run_bass_kernel_spmd

T, D, I, E, TOPK = 2048, 1024, 4096, 8, 2
N_CORES = 8
CAP = 512            # token tile capacity (multiple of 128, <=512)
N_TOK = 504          # matmul free dim: tokens actually computed per pass
DT = D // 128        # 8 contraction steps for layer 1
ND = D // 128        # 8 output d-tiles for layer 2
NI = I // 128        # 32 intermediate tiles
NF = 4               # first-chunk dup: w1 i-tile 0, dt chunks 0..NF-1
N_WARM = 20          # dummy matmuls to lift the HAM clock gate at startup
F32 = mybir.dt.float32
BF16 = mybir.dt.bfloat16
NP_BF16 = mybir.dt.np(BF16)

_NC = None           # compiled Bass module, built once per process
_WCACHE = {}         # packed per-expert weights, keyed on input identity
LAST_RESULTS = None  # BassKernelResults of the most recent SPMD run


def _build_nc(sim_act=False):
    # sim_act: CoreSim lacks Silu; emit sigmoid + extra multiply instead
    # (same math) so the program can be validated in simulation.
    nc = bacc.Bacc(
        "TRN2", target_bir_lowering=False, debug=False, num_devices=N_CORES
    )
    # Packed layouts (see _pack_weights): every DMA below is a contiguous
    # HBM range.
    xt1_d = nc.dram_tensor("xt1", [DT, 128, CAP], BF16, kind="ExternalInput").ap()
    g_d = nc.dram_tensor("g", [1, CAP], BF16, kind="ExternalInput").ap()
    w13f_d = nc.dram_tensor("w13f", [NF, 128, 128], BF16, kind="ExternalInput").ap()
    w13p_d = nc.dram_tensor(
        "w13p", [NI, 2, 128, DT, 128], BF16, kind="ExternalInput"
    ).ap()
    w2t_d = nc.dram_tensor("w2t", [I, D], BF16, kind="ExternalInput").ap()
    y_d = nc.dram_tensor("y", [D, CAP], BF16, kind="ExternalOutput").ap()

    with tile.TileContext(nc) as tc:
        with (
            tc.tile_pool(name="consts", bufs=1) as const_pool,
            tc.tile_pool(name="w13", bufs=10) as w13_pool,
            tc.tile_pool(name="h", bufs=1) as h_pool,
            tc.tile_pool(name="tmp", bufs=2) as tmp_pool,
            tc.tile_pool(name="yout", bufs=4) as out_pool,
        ):
            # HAM warmup + gate-broadcast seeds (no DMA deps except tiny g).
            warm_sb = const_pool.tile([128, 128], BF16)
            nc.vector.memset(warm_sb[:], 0.0)
            ones_sb = const_pool.tile([1, 128], BF16)
            nc.vector.memset(ones_sb[:], 1.0)
            g_row = const_pool.tile([1, CAP], BF16)

            # Startup-critical loads, in issue order. The first matmul gates
            # on one 32 KB w13f chunk + the 128 KB xt1 d-tile 0.
            xt1_sb = const_pool.tile([128, DT, CAP], BF16)
            xt3_sb = const_pool.tile([128, DT, CAP], BF16)
            w13_t0 = w13_pool.tile([128, 2, DT, 128], BF16, tag="w13")
            nc.sync.dma_start(g_row[:], g_d)
            nc.sync.dma_start(xt1_sb[:, 0, :], xt1_d[0])
            for dt_i in range(NF):
                nc.sync.dma_start(w13_t0[:, 0, dt_i, :], w13f_d[dt_i])
            # h3's m=1 plane (256 KB linear), then the strided second half of
            # the m=0 plane (dt chunks NF.. were not duplicated).
            nc.sync.dma_start(w13_t0[:, 1], w13p_d[0, 1])
            nc.sync.dma_start(w13_t0[:, 0, NF:, :], w13p_d[0, 0, :, NF:, :])
            for dt_i in range(1, DT):
                nc.sync.dma_start(xt1_sb[:, dt_i, :], xt1_d[dt_i])

            # hT[i, t] — layer-1 output (gate folded in via the x3 branch),
            # transposed so it is the layer-2 moving operand directly.
            hT = h_pool.tile([128, NI, CAP], BF16)

            # w2 lives fully in SBUF (64 KB/partition bf16); its i-tiles are
            # streamed in during layer 1 so layer 2 never waits on HBM.
            w2_sb = const_pool.tile([128, NI, D], BF16)
            w2t_r = w2t_d.rearrange("(a p) d -> p a d", p=128)

            # Warmup matmuls, then the gate outer product: g_bc[p, t] =
            # ones[p] * g[t] via a K=1 matmul, copied to SBUF by DVE (ACT is
            # busy with its activation-table loads at startup).
            warm_pool = tc.alloc_tile_pool(name="warm", bufs=1, space="PSUM")
            warm_ps = warm_pool.tile([128, 128], F32)
            for _ in range(N_WARM):
                nc.tensor.matmul(
                    warm_ps[:], warm_sb[:], warm_sb[:], start=True, stop=True
                )
            gps_pool = tc.alloc_tile_pool(name="gps", bufs=1, space="PSUM")
            g_ps = gps_pool.tile([128, CAP], F32)
            nc.tensor.matmul(g_ps[:], ones_sb[:], g_row[:], start=True, stop=True)
            g_bc = const_pool.tile([128, CAP], BF16)
            nc.vector.tensor_copy(g_bc[:], g_ps[:])
            gps_pool.release()
            warm_pool.release()
            # x3 = g * x, built per d-tile as its xt1 slice lands.
            for dt_i in range(DT):
                nc.vector.tensor_mul(
                    xt3_sb[:, dt_i, :N_TOK],
                    xt1_sb[:, dt_i, :N_TOK],
                    g_bc[:, :N_TOK],
                )

            ps1_pool = tc.alloc_tile_pool(name="ps1", bufs=2, space="PSUM")
            ps2_pool = tc.alloc_tile_pool(name="ps2", bufs=2, space="PSUM")
            for it in range(NI):
                if it == 0:
                    w13_t = w13_t0
                else:
                    w13_t = w13_pool.tile([128, 2, DT, 128], BF16, tag="w13")
                    for m in range(2):
                        nc.sync.dma_start(w13_t[:, m], w13p_d[it, m])
                # Stream one w2 i-tile per iteration into the resident tile,
                # delayed so it never queues ahead of ramp-critical w13.
                if it >= 6:
                    nc.sync.dma_start(w2_sb[:, it - 6, :], w2t_r[:, it - 6, :])
                w1_t = w13_t[:, 0]
                w3_t = w13_t[:, 1]

                h1_ps = ps1_pool.tile([128, CAP], F32, tag="h1")
                h3_ps = ps1_pool.tile([128, CAP], F32, tag="h3")
                for dt_i in range(DT):
                    nc.tensor.matmul(
                        h1_ps[:, :N_TOK],
                        w1_t[:, dt_i, :],
                        xt1_sb[:, dt_i, :N_TOK],
                        start=(dt_i == 0),
                        stop=(dt_i == DT - 1),
                    )
                for dt_i in range(DT):
                    nc.tensor.matmul(
                        h3_ps[:, :N_TOK],
                        w3_t[:, dt_i, :],
                        xt3_sb[:, dt_i, :N_TOK],
                        start=(dt_i == 0),
                        stop=(dt_i == DT - 1),
                    )
                s_sb = tmp_pool.tile([128, N_TOK], F32)
                if sim_act:
                    nc.scalar.activation(
                        s_sb[:], h1_ps[:, :N_TOK],
                        mybir.ActivationFunctionType.Sigmoid,
                    )
                    nc.vector.tensor_mul(s_sb[:], s_sb[:], h1_ps[:, :N_TOK])
                else:
                    nc.scalar.activation(
                        s_sb[:], h1_ps[:, :N_TOK],
                        mybir.ActivationFunctionType.Silu,
                    )
                nc.vector.tensor_mul(
                    hT[:, it, :N_TOK], s_sb[:], h3_ps[:, :N_TOK]
                )
            # Remaining w2 i-tiles (land well before layer 2 needs them).
            for it in range(NI - 6, NI):
                nc.sync.dma_start(w2_sb[:, it, :], w2t_r[:, it, :])

            # Layer 2, d-tile-major, transposed: yT[d, t] accumulates over
            # all 32 i-tiles into 1 PSUM bank per d-tile; drain is a single
            # ACT/DVE copy + one linear 128 KB DMA, pipelined 2 deep.
            for dd in range(ND):
                yT_ps = ps2_pool.tile([128, 512], F32, tag="yt")
                for it in range(NI):
                    nc.tensor.matmul(
                        yT_ps[:, :N_TOK],
                        w2_sb[:, it, dd * 128:(dd + 1) * 128],
                        hT[:, it, :N_TOK],
                        start=(it == 0),
                        stop=(it == NI - 1),
                    )
                yT_sb = out_pool.tile([128, CAP], BF16)
                if dd == ND - 1:
                    # Exposed tail: split the drain across both engines.
                    half = N_TOK // 2
                    nc.scalar.activation(
                        yT_sb[:, :half], yT_ps[:, :half],
                        mybir.ActivationFunctionType.Copy,
                    )
                    nc.vector.tensor_copy(
                        yT_sb[:, half:N_TOK], yT_ps[:, half:N_TOK]
                    )
                    for h in range(2):
                        nc.sync.dma_start(
                            y_d[dd * 128 + h * 64:dd * 128 + (h + 1) * 64, :],
                            yT_sb[h * 64:(h + 1) * 64, :],
                        )
                else:
                    if dd % 2 == 0:
                        nc.scalar.activation(
                            yT_sb[:, :N_TOK], yT_ps[:, :N_TOK],
                            mybir.ActivationFunctionType.Copy,
                        )
                    else:
                        nc.vector.tensor_copy(yT_sb[:, :N_TOK], yT_ps[:, :N_TOK])
                    nc.sync.dma_start(
                        y_d[dd * 128:(dd + 1) * 128, :], yT_sb[:]
                    )
            ps2_pool.release()
            ps1_pool.release()

    nc.compile()
    return nc


def _pack_weights(w1, w2, w3):
    """Per-expert device layouts (bf16), all contiguous HBM reads:
    w13p[it, m, p, dt, c] = w_m[it*128+c, dt*128+p]  (w.T tiled for lhsT;
    each (it, m) plane is one contiguous 256 KB read, 2 KB per partition)
    w13f[dt, p, c] = w13p[0, 0, p, dt, c] for dt < NF  (startup dup chunks)
    w2t = w2.T ([I, D], i rows on partitions)."""
    key = tuple((a.ctypes.data, a.shape) for a in (w1, w2, w3))
    if _WCACHE.get("key") == key:
        return _WCACHE["maps"]
    maps = []
    for e in range(E):
        w13p = np.empty((NI, 2, 128, DT, 128), dtype=NP_BF16)
        # w[e] is [I, D]: reshape [it, c, dt, p] -> transpose to [it, p, dt, c]
        w13p[:, 0] = w1[e].reshape(NI, 128, DT, 128).transpose(0, 3, 2, 1).astype(NP_BF16)
        w13p[:, 1] = w3[e].reshape(NI, 128, DT, 128).transpose(0, 3, 2, 1).astype(NP_BF16)
        w13f = np.ascontiguousarray(w13p[0, 0].transpose(1, 0, 2)[:NF])
        w2t = np.ascontiguousarray(w2[e].T).astype(NP_BF16)
        maps.append({"w13p": w13p, "w13f": w13f, "w2t": w2t})
    _WCACHE["key"] = key
    _WCACHE["maps"] = maps
    return maps


def kernel(x, expert_indices, expert_weights, w1, w2, w3):
    global _NC, LAST_RESULTS
    x = np.ascontiguousarray(np.asarray(x, dtype=np.float32))
    idx = np.asarray(expert_indices)
    ew = np.asarray(expert_weights, dtype=np.float32)
    w1 = np.ascontiguousarray(np.asarray(w1, dtype=np.float32))
    w2 = np.ascontiguousarray(np.asarray(w2, dtype=np.float32))
    w3 = np.ascontiguousarray(np.asarray(w3, dtype=np.float32))

    if _NC is None:
        _NC = _build_nc()

    # Host routing: unique tokens per expert, with both top-k gate weights of a
    # token merged (a token picking the same expert twice gets the summed gate).
    tok_lists, gate_lists = [], []
    for e in range(E):
        m = idx == e
        sel = np.nonzero(m.any(axis=1))[0]
        tok_lists.append(sel)
        gate_lists.append((ew * m).sum(axis=1)[sel].astype(np.float32))

    weight_maps = _pack_weights(w1, w2, w3)

    n_pass = max(1, math.ceil(max(len(s) for s in tok_lists) / N_TOK))
    out = np.zeros((T, D), dtype=np.float32)
    trace = bool(os.environ.get("BASS_TRACE"))
    for p in range(n_pass):
        in_maps = []
        chunks = []
        for e in range(E):
            sel = tok_lists[e][p * N_TOK:(p + 1) * N_TOK]
            g = gate_lists[e][p * N_TOK:(p + 1) * N_TOK]
            chunks.append(sel)
            xt1 = np.zeros((DT, 128, CAP), dtype=NP_BF16)
            g_pad = np.zeros((1, CAP), dtype=NP_BF16)
            if len(sel):
                xt1.reshape(D, CAP)[:, :len(sel)] = x[sel].T.astype(NP_BF16)
                g_pad[0, :len(sel)] = g.astype(NP_BF16)
            in_maps.append({"xt1": xt1, "g": g_pad, **weight_maps[e]})
        # Rare transient NRT_EXEC_UNIT_UNRECOVERABLE errors have been observed
        # on the first execution of a fresh NEFF; a straight retry recovers.
        last_exc = None
        for attempt in range(3):
            try:
                LAST_RESULTS = run_bass_kernel_spmd(
                    _NC, in_maps, core_ids=list(range(N_CORES)),
                    trace=trace and attempt == 0,
                )
                break
            except Exception as exc:  # noqa: BLE001
                last_exc = exc
                time.sleep(3)
        else:
            raise last_exc
        for e in range(E):
            sel = chunks[e]
            if len(sel):
                out[sel] += (
                    LAST_RESULTS.results[e]["y"][:, :len(sel)]
                    .T.astype(np.float32)
                )
    return out
